# revision 11
# baseline (speedup 1.0000x reference)
"""Trainium2 Bass kernel for ConvPosMultiHeadAttn_Order.

Sharding: 8 cores = (batch b in 0..3) x (head-group hg in 0..1), 8 heads/core.

All matmul operands bf16 (fp32 PSUM accumulate). Causal slicing of the
score/PV matmuls and exp to the live [r:512] column range. Two-stage q-mask
evictions (ACT psum->bf16 copy, then 2x-rate bf16 DVE muls against
mask tiles MCM/MCM2 -- two layouts so both DVE inputs share a start
partition). Key blends write psum->bf16 tiles directly (one ACT copy + two
DVE copy_predicated, one of them SBUF-sourced). Emission is interleaved at
(proj nt-half, attn qt-half) granularity -- attn qt=0 only needs the nt=0
half of the projections -- which shortens dependency horizons and lets the
engines overlap deeply. GPSIMD (Pool) runs the causal affine_selects only
(it cannot touch PSUM); PSUM evictions are split between ACT and DVE.

Per-core decomposition:
  - x^T resident in SBUF; transposed projections with HOST-side weight column
    layouts:
      * Q pair lhsT (even h) = [Wq_h | Wq_h+1] -> PSUM [q_h; q_h+1]
      * K lhsT               = [Wk2_h | Wk1_h] -> PSUM [k2_h; k1_h]
    plus pe-table projections for the relative-position terms.
  - Speaker-select folded into an extended 256-dim score contraction:
      score^T[k,q] = [q*sq; q*(1-sq)] . [KA; KB] + [qp*sq; qp*(1-sq)] . [KPA; KPB]
    where KA = sk?k1:k2, KB = sk?k2:k1 (ACT copy + DVE copy_predicated from
    the interleaved PSUM into bf16 key tiles), q-side masks applied by DVE
    bf16 muls after an ACT psum->bf16 eviction.
  - Causal: score matmuls and exp sliced to [r:512] on diagonal tiles; one
    gpsimd affine_select (full width) zeroes k > q and the unwritten prefix.
  - Softmax denominators: ones-column (scaled by umask) appended to V in the
    PV lhsT -> row 64 of the PV PSUM holds the per-query sums. umask also
    scales V rows (exactly reproduces the reference key masking).
  - Normalize via reciprocal + PE outer-product broadcast, written shifted
    into the packed FC lhsT; final FC matmul + DMA out.
Host sums the two head-group partial outputs per batch.
"""
import sys

sys.path.insert(0, "/opt/trn_rl_repo")

import numpy as np

D = 1024
L = 1024
B = 4
DH = 64
NH = 8          # heads per core
NCORES = 8

_cached = {}


def _pe_table():
    num = 1201
    half = DH // 2
    freq = np.exp(np.arange(half, dtype=np.float32) * (-np.log(10000.0) / (half - 1)))
    pos_vals = np.arange(-num // 2, num // 2, dtype=np.float32)
    ang = pos_vals[:, None] * freq[None, :]
    table = np.concatenate([np.sin(ang), np.cos(ang)], axis=1).astype(np.float32)
    table[0] = 0.0
    idx = np.arange(-(L // 2), L // 2) + (num // 2 + 1)
    return table[idx]  # [L, DH] float32


def _build_program(nrep=1, loop=None):
    import concourse.bass as bass
    import concourse.mybir as mybir
    import concourse.tile as tile
    from concourse import bacc

    f32 = mybir.dt.float32
    f32r = mybir.dt.float32r
    bf16 = mybir.dt.bfloat16
    u8 = mybir.dt.uint8
    Exp = mybir.ActivationFunctionType.Exp
    Copy = mybir.ActivationFunctionType.Copy

    nc = bacc.Bacc(None, target_bir_lowering=False, debug=False)

    XT = nc.declare_dram_parameter("XT", [D, L], bf16, isOutput=False)
    WQK = nc.declare_dram_parameter("WQK", [NH, 2, D, 128], bf16, isOutput=False)
    WPOS = nc.declare_dram_parameter("WPOS", [NH, 2, DH, 128], bf16, isOutput=False)
    WV = nc.declare_dram_parameter("WV", [D, 512], bf16, isOutput=False)
    WFC = nc.declare_dram_parameter("WFC", [512, D], bf16, isOutput=False)
    PET = nc.declare_dram_parameter("PET", [DH, L], bf16, isOutput=False)
    MCM = nc.declare_dram_parameter("MCM", [128, L], bf16, isOutput=False)
    MCM2 = nc.declare_dram_parameter("MCM2", [128, L], bf16, isOutput=False)
    SKM = nc.declare_dram_parameter("SKM", [128, L], u8, isOutput=False)
    ONES1 = nc.declare_dram_parameter("ONES1", [1, 128], f32r, isOutput=False)
    UMASKT = nc.declare_dram_parameter("UMASKT", [128, 8], f32, isOutput=False)
    OCOLREP = nc.declare_dram_parameter("OCOLREP", [128, 64], bf16, isOutput=False)
    Y = nc.declare_dram_parameter("Y", [L, D], f32, isOutput=True)

    with tile.TileContext(nc) as tc:
        with tc.tile_pool(name="const", bufs=1) as const, \
             tc.tile_pool(name="wstream", bufs=3) as wstream, \
             tc.tile_pool(name="qk2", bufs=2) as qk2, \
             tc.tile_pool(name="qk3", bufs=3) as qk3, \
             tc.tile_pool(name="qev", bufs=3) as qev, \
             tc.tile_pool(name="exps", bufs=12) as exps, \
             tc.tile_pool(name="small", bufs=2) as small, \
             tc.tile_pool(name="yt", bufs=4) as ytp, \
             tc.tile_pool(name="proj_ps", bufs=3, space="PSUM") as proj_ps, \
             tc.tile_pool(name="score_ps", bufs=3, space="PSUM") as score_ps, \
             tc.tile_pool(name="pv_ps", bufs=2, space="PSUM") as pv_ps:

            # ---- resident constants; DMAs spread across engine queues and
            # ordered so head-0 can start ASAP ----
            xt = []
            for k in range(8):
                t = const.tile([128, L], bf16, tag=f"xt{k}")
                xt.append(t)
            dmaq = [nc.sync, nc.gpsimd, nc.sync, nc.sync]
            import contextlib
            loop_ctx = tc.For_i(0, loop, 1) if loop else contextlib.nullcontext()
            with loop_ctx:
              for _rep in range(nrep):
                  wpre = {}
                  wq0 = wstream.tile([128, D], bf16, tag="wq")
                  nc.sync.dma_start(wq0[:].rearrange("p (k c) -> p k c", c=128),
                                    WQK[0, 0].rearrange("(k p) c -> p k c", p=128))
                  wpre[("wq", 0)] = wq0
                  wk0 = wstream.tile([128, D], bf16, tag="wk")
                  nc.gpsimd.dma_start(wk0[:].rearrange("p (k c) -> p k c", c=128),
                                      WQK[0, 1].rearrange("(k p) c -> p k c", p=128))
                  wpre[("wk", 0)] = wk0
                  nc.sync.dma_start(xt[0][:], XT[0:128, :])
                  nc.sync.dma_start(xt[1][:], XT[128:256, :])
                  for k in range(2, 8):
                      dmaq[k % 4].dma_start(xt[k][:], XT[k * 128:(k + 1) * 128, :])
                  pet = const.tile([DH, L], bf16, tag="pet")
                  nc.gpsimd.dma_start(pet[:], PET[:])
                  mcm = const.tile([128, L], bf16, tag="mcm")
                  nc.sync.dma_start(mcm[:], MCM[:])
                  mcm2 = const.tile([128, L], bf16, tag="mcm2")
                  nc.sync.dma_start(mcm2[:], MCM2[:])
                  skm = const.tile([128, L], u8, tag="skm")
                  nc.gpsimd.dma_start(skm[:], SKM[:])
                  wqp, wkp = [], []
                  for h in range(NH):
                      if h % 2 == 0:
                          t0 = const.tile([DH, 128], bf16, tag=f"wqp{h}")
                          dmaq[h % 4].dma_start(t0[:], WPOS[h, 0])
                          wqp.append(t0)
                      else:
                          wqp.append(None)
                      t1 = const.tile([DH, 128], bf16, tag=f"wkp{h}")
                      dmaq[(h + 2) % 4].dma_start(t1[:], WPOS[h, 1])
                      wkp.append(t1)
                  ones1 = const.tile([1, 128], f32r, tag="ones1")
                  nc.sync.dma_start(ones1[:], ONES1[:])
                  umaskt = const.tile([128, 8], f32, tag="umaskt")
                  nc.sync.dma_start(umaskt[:], UMASKT[:])
                  vext = []
                  for tcn in range(8):
                      t = const.tile([128, NH * 65], bf16, tag=f"vext{tcn}")
                      vext.append(t)
                  outn = []
                  for g in range(4):
                      t = const.tile([128, L], bf16, tag=f"outn{g}")
                      outn.append(t)

                  hstate = {}
                  wcur = {}

                  def emit_proj_nt(h, nt):
                      ntsl = bass.ts(nt, 512)
                      if h % 2 == 0:
                          if nt == 0:
                              if ("wq", h) in wpre:
                                  wcur[("wq", h)] = wpre.pop(("wq", h))
                              else:
                                  wq_t = wstream.tile([128, D], bf16, tag="wq")
                                  nc.sync.dma_start(
                                      wq_t[:].rearrange("p (k c) -> p k c", c=128),
                                      WQK[h, 0].rearrange("(k p) c -> p k c", p=128))
                                  wcur[("wq", h)] = wq_t
                              qsd0 = qk3.tile([128, L], bf16, tag="qsd")
                              qsd1 = qk3.tile([128, L], bf16, tag="qsd")
                              qpsd0 = qk3.tile([128, L], bf16, tag="qpsd")
                              qpsd1 = qk3.tile([128, L], bf16, tag="qpsd")
                              hstate[("q", h)] = (qsd0, qpsd0)
                              hstate[("q", h + 1)] = (qsd1, qpsd1)
                          wq_t = wcur[("wq", h)]
                          qsd0, qpsd0 = hstate[("q", h)]
                          qsd1, qpsd1 = hstate[("q", h + 1)]
                          psq = proj_ps.tile([128, 512], f32, tag="proj")
                          for k in range(8):
                              nc.tensor.matmul(psq[:], wq_t[:, k * 128:(k + 1) * 128],
                                               xt[k][:, ntsl],
                                               start=(k == 0), stop=(k == 7))
                          qh = qev.tile([128, 512], bf16, tag="qh")
                          nc.scalar.copy(qh[:], psq[:])
                          nc.vector.tensor_mul(qsd0[0:64, ntsl], qh[0:64, :],
                                               mcm[0:64, ntsl])
                          nc.vector.tensor_mul(qsd0[64:128, ntsl], qh[0:64, :],
                                               mcm2[0:64, ntsl])
                          nc.vector.tensor_mul(qsd1[0:64, ntsl], qh[64:128, :],
                                               mcm2[64:128, ntsl])
                          nc.vector.tensor_mul(qsd1[64:128, ntsl], qh[64:128, :],
                                               mcm[64:128, ntsl])
                          psqp = proj_ps.tile([128, 512], f32, tag="proj")
                          nc.tensor.matmul(psqp[:], wqp[h][:], pet[:, ntsl],
                                           start=True, stop=True)
                          qph = qev.tile([128, 512], bf16, tag="qph")
                          nc.scalar.copy(qph[:], psqp[:])
                          nc.vector.tensor_mul(qpsd0[0:64, ntsl], qph[0:64, :],
                                               mcm[0:64, ntsl])
                          nc.vector.tensor_mul(qpsd0[64:128, ntsl], qph[0:64, :],
                                               mcm2[0:64, ntsl])
                          nc.vector.tensor_mul(qpsd1[0:64, ntsl], qph[64:128, :],
                                               mcm2[64:128, ntsl])
                          nc.vector.tensor_mul(qpsd1[64:128, ntsl], qph[64:128, :],
                                               mcm[64:128, ntsl])

                      # K / KP interleaved projections + blends (per head)
                      if nt == 0:
                          if ("wk", h) in wpre:
                              wcur[("wk", h)] = wpre.pop(("wk", h))
                          else:
                              wk_t = wstream.tile([128, D], bf16, tag="wk")
                              nc.sync.dma_start(
                                  wk_t[:].rearrange("p (k c) -> p k c", c=128),
                                  WQK[h, 1].rearrange("(k p) c -> p k c", p=128))
                              wcur[("wk", h)] = wk_t
                          k1t = qk2.tile([128, L], bf16, tag="k1t")
                          k2t = qk2.tile([128, L], bf16, tag="k2t")
                          hstate[("k", h)] = (k1t, k2t)
                      wk_t = wcur[("wk", h)]
                      k1t, k2t = hstate[("k", h)]
                      psk = proj_ps.tile([128, 512], f32, tag="proj")
                      for k in range(8):
                          nc.tensor.matmul(psk[:], wk_t[:, k * 128:(k + 1) * 128],
                                           xt[k][:, ntsl], start=(k == 0), stop=(k == 7))
                      nc.scalar.copy(k1t[:, ntsl], psk[:])
                      nc.vector.copy_predicated(k1t[0:64, ntsl], skm[64:128, ntsl],
                                                k1t[64:128, ntsl])
                      nc.vector.copy_predicated(k1t[64:128, ntsl], skm[64:128, ntsl],
                                                psk[0:64, :])

                      pskp = proj_ps.tile([128, 512], f32, tag="proj")
                      nc.tensor.matmul(pskp[:], wkp[h][:], pet[:, ntsl],
                                       start=True, stop=True)
                      nc.scalar.copy(k2t[:, ntsl], pskp[:])
                      nc.vector.copy_predicated(k2t[0:64, ntsl], skm[64:128, ntsl],
                                                k2t[64:128, ntsl])
                      nc.vector.copy_predicated(k2t[64:128, ntsl], skm[64:128, ntsl],
                                                pskp[0:64, :])

                  def emit_attn_qt(h, qt):
                      qsd, qpsd = hstate[("q", h)]
                      k1t, k2t = hstate[("k", h)]
                      qtsl = bass.ts(qt, 512)
                      jmax = 4 * (qt + 1)
                      ets = []
                      for j in range(jmax):
                          sps = score_ps.tile([128, 512], f32, tag="s")
                          et = exps.tile([128, 512], bf16, tag="e")
                          r = j * 128 - qt * 512
                          if r < 0:
                              nc.tensor.matmul(sps[:], k1t[:, j * 128:(j + 1) * 128],
                                               qsd[:, qtsl], start=True, stop=False)
                              nc.tensor.matmul(sps[:], k2t[:, j * 128:(j + 1) * 128],
                                               qpsd[:, qtsl], start=False, stop=True)
                              nc.scalar.activation(et[:], sps[:], Exp)
                          else:
                              qs = qt * 512
                              nc.tensor.matmul(sps[:, r:512],
                                               k1t[:, j * 128:(j + 1) * 128],
                                               qsd[:, qs + r:qs + 512],
                                               start=True, stop=False)
                              nc.tensor.matmul(sps[:, r:512],
                                               k2t[:, j * 128:(j + 1) * 128],
                                               qpsd[:, qs + r:qs + 512],
                                               start=False, stop=True)
                              nc.scalar.activation(et[:, r:512], sps[:, r:512], Exp)
                              # keep where y - x - r >= 0 (q >= k), else 0
                              nc.gpsimd.affine_select(
                                  out=et[:], in_=et[:],
                                  compare_op=mybir.AluOpType.is_ge,
                                  fill=0.0, base=-r,
                                  pattern=[[1, 512]], channel_multiplier=-1)
                          ets.append(et)
                      pvps = pv_ps.tile([65, 512], f32, tag="pv")
                      for j in range(jmax):
                          r = max(j * 128 - qt * 512, 0)
                          nc.tensor.matmul(pvps[:, r:512],
                                           vext[j][:, h * 65:(h + 1) * 65],
                                           ets[j][:, r:512],
                                           start=(j == 0), stop=(j == jmax - 1))
                      rc = small.tile([1, 512], f32, tag="rc")
                      nc.vector.reciprocal(rc[:], pvps[64:65, :])
                      rcr = small.tile([1, 512], f32r, tag="rcr")
                      nc.vector.tensor_copy(rcr[:], rc[:])
                      hstate[("n", h, qt)] = (pvps, rcr)

                  def emit_norm(h):
                      hstate.pop(("q", h))
                      hstate.pop(("k", h))
                      for qt in range(2):
                          pvps, rcr = hstate.pop(("n", h, qt))
                          qtsl = bass.ts(qt, 512)
                          bps = score_ps.tile([64, 512], f32, tag="s")
                          nc.tensor.matmul(bps[:], ones1[:, 0:64], rcr[:],
                                           start=True, stop=True)
                          bsb = small.tile([64, 512], f32, tag="bsb")
                          nc.scalar.copy(bsb[:], bps[:])
                          g, row0 = h // 2, (h % 2) * 64
                          nc.vector.tensor_mul(outn[g][row0:row0 + 64, qtsl],
                                               pvps[0:64, :], bsb[:])

                  emit_proj_nt(0, 0)
                  emit_proj_nt(0, 1)

                  # ---- V phase (needs xt + wv; emitted after proj(0) so head-0
                  # scores are not delayed behind the wv DMA) ----
                  wv = []
                  for k in range(8):
                      t = const.tile([128, 512], bf16, tag=f"wv{k}")
                      dmaq[k % 4].dma_start(t[:], WV[k * 128:(k + 1) * 128, :])
                      wv.append(t)
                  ocolrep = const.tile([128, 64], bf16, tag="ocolrep")
                  nc.sync.dma_start(ocolrep[:], OCOLREP[:])
                  for tcn in range(8):
                      ocols = vext[tcn][:].rearrange("p (h c) -> p h c", c=65)[:, :, 64]
                      nc.sync.dma_start(ocols, OCOLREP[:, tcn * 8:(tcn + 1) * 8])
                      pool = proj_ps if tcn % 2 == 0 else score_ps
                      psv = pool.tile([128, 512], f32, tag="proj" if tcn % 2 == 0 else "s")
                      for k in range(8):
                          nc.tensor.matmul(psv[:], xt[k][:, tcn * 128:(tcn + 1) * 128],
                                           wv[k][:], start=(k == 0), stop=(k == 7))
                      vslots = vext[tcn][:].rearrange(
                          "p (h c) -> p h c", c=65)[:, :, 0:64]
                      nc.scalar.activation(
                          vslots, psv[:].rearrange("p (h c) -> p h c", c=64),
                          Copy, scale=umaskt[:, tcn:tcn + 1])

                  for h in range(1, NH):
                      emit_proj_nt(h, 0)
                      emit_attn_qt(h - 1, 0)
                      emit_proj_nt(h, 1)
                      emit_attn_qt(h - 1, 1)
                      emit_norm(h - 1)
                  # wfc loads start as soon as the last head's W slots free up
                  wfc = []
                  for kc in range(4):
                      t = wstream.tile([128, D], bf16, tag=("wq" if kc % 2 == 0 else "wk"))
                      nc.sync.dma_start(t[:], WFC[kc * 128:(kc + 1) * 128, :])
                      wfc.append(t)
                  emit_attn_qt(NH - 1, 0)
                  emit_attn_qt(NH - 1, 1)
                  emit_norm(NH - 1)

                  # ---- FC (alternate PSUM pools to avoid eviction stalls) ----
                  for tcn in range(8):
                      tsl = bass.ts(tcn, 128)
                      for ct in range(2):
                          ctsl = bass.ts(ct, 512)
                          i3 = (tcn * 2 + ct) % 3
                          pool = (score_ps, proj_ps, pv_ps)[i3]
                          yps = pool.tile([128, 512], f32, tag=("s", "proj", "pv")[i3])
                          for kc in range(4):
                              nc.tensor.matmul(yps[:], outn[kc][:, tsl],
                                               wfc[kc][:, ctsl],
                                               start=(kc == 0), stop=(kc == 3))
                          yt = ytp.tile([128, 512], f32, tag="y")
                          if (tcn * 2 + ct) % 2 == 0:
                              nc.vector.tensor_copy(yt[:], yps[:])
                          else:
                              nc.scalar.copy(yt[:], yps[:])
                          dmaq[(tcn * 2 + ct) % 4].dma_start(
                              Y[tcn * 128:(tcn + 1) * 128, ct * 512:(ct + 1) * 512], yt[:])

    nc.compile()
    return nc


def _host_inputs(embed, umask, qmask, W_qkv, W_pos, W_fc):
    import ml_dtypes
    bf16 = ml_dtypes.bfloat16
    pe = _pe_table()
    pet = np.ascontiguousarray(pe.T).astype(bf16)  # [DH, L]
    ones1 = np.ones((1, 128), np.float32)
    in_maps = []
    for core in range(NCORES):
        b, hg = core // 2, core % 2
        sq = qmask[b].astype(np.float32)          # [L] in {0,1}
        um = umask[b].astype(np.float32)          # [L]
        mcm = np.empty((128, L), np.float32)
        mcm[0:64] = sq[None, :]
        mcm[64:128] = (1.0 - sq)[None, :]
        mcm2 = np.empty((128, L), np.float32)
        mcm2[0:64] = (1.0 - sq)[None, :]
        mcm2[64:128] = sq[None, :]
        skm = np.broadcast_to(qmask[b].astype(np.uint8)[None, :], (128, L)).copy()
        wqk = np.zeros((NH, 2, D, 128), np.float32)
        wpos = np.zeros((NH, 2, DH, 128), np.float32)
        for h in range(NH):
            gh = hg * NH + h
            k1c = W_qkv[:, 1 * D + gh * DH: 1 * D + (gh + 1) * DH]
            k2c = W_qkv[:, 2 * D + gh * DH: 2 * D + (gh + 1) * DH]
            wqk[h, 1] = np.concatenate([k2c, k1c], axis=1)
            kp1c = W_pos[:, 1 * D + gh * DH: 1 * D + (gh + 1) * DH]
            kp2c = W_pos[:, 2 * D + gh * DH: 2 * D + (gh + 1) * DH]
            wpos[h, 1] = np.concatenate([kp2c, kp1c], axis=1)
            if h % 2 == 0:
                q0 = W_qkv[:, 0 * D + gh * DH: 0 * D + (gh + 1) * DH]
                q1 = W_qkv[:, 0 * D + (gh + 1) * DH: 0 * D + (gh + 2) * DH]
                wqk[h, 0] = np.concatenate([q0, q1], axis=1)
                qp0 = W_pos[:, 0 * D + gh * DH: 0 * D + (gh + 1) * DH]
                qp1 = W_pos[:, 0 * D + (gh + 1) * DH: 0 * D + (gh + 2) * DH]
                wpos[h, 0] = np.concatenate([qp0, qp1], axis=1)
        umaskt = um.reshape(8, 128).T.copy()                     # [128, 8]
        ocolrep = np.repeat(umaskt[:, :, None], 8, axis=2).reshape(128, 64)
        in_maps.append({
            "XT": np.ascontiguousarray(embed[b].T).astype(bf16),
            "WQK": wqk.astype(bf16),
            "WPOS": wpos.astype(bf16),
            "WV": np.ascontiguousarray(
                W_qkv[:, 3 * D + hg * 512: 3 * D + (hg + 1) * 512]).astype(bf16),
            "WFC": np.ascontiguousarray(W_fc[hg * 512:(hg + 1) * 512, :]).astype(bf16),
            "PET": pet,
            "MCM": mcm.astype(bf16),
            "MCM2": mcm2.astype(bf16),
            "SKM": skm,
            "ONES1": ones1,
            "UMASKT": umaskt,
            "OCOLREP": np.ascontiguousarray(ocolrep).astype(bf16),
        })
    return in_maps


def kernel(embed, umask, qmask, W_qkv, W_pos, W_fc):
    from concourse.bass_utils import run_bass_kernel_spmd

    embed = np.asarray(embed, dtype=np.float32)
    umask = np.asarray(umask)
    qmask = np.asarray(qmask)
    W_qkv = np.asarray(W_qkv, dtype=np.float32)
    W_pos = np.asarray(W_pos, dtype=np.float32)
    W_fc = np.asarray(W_fc, dtype=np.float32)

    if "nc" not in _cached:
        _cached["nc"] = _build_program()
    nc = _cached["nc"]

    in_maps = _host_inputs(embed, umask, qmask, W_qkv, W_pos, W_fc)
    res = run_bass_kernel_spmd(nc, in_maps, list(range(NCORES))).results

    y = np.empty((B, L, D), np.float32)
    for b in range(B):
        y[b] = res[2 * b]["Y"] + res[2 * b + 1]["Y"]
    return y


# revision 12
# speedup vs baseline: 1.5551x; 1.5551x over previous
"""Trainium2 Bass kernel for ConvPosMultiHeadAttn_Order.

Sharding: 8 cores = (batch b in 0..3) x (head-group hg in 0..1), 8 heads/core.

All matmul operands bf16 (fp32 PSUM accumulate). Causal slicing of the
score/PV matmuls and exp to the live [r:512] column range. Two-stage q-mask
evictions (ACT psum->bf16 copy, then 2x-rate bf16 DVE muls against
mask tiles MCM/MCM2 -- two layouts so both DVE inputs share a start
partition). Key blends write psum->bf16 tiles directly (one ACT copy + two
DVE copy_predicated, one of them SBUF-sourced). Emission is interleaved at
(proj nt-half, attn qt-half) granularity -- attn qt=0 only needs the nt=0
half of the projections -- which shortens dependency horizons and lets the
engines overlap deeply. GPSIMD (Pool) runs the causal affine_selects only
(it cannot touch PSUM); PSUM evictions are split between ACT and DVE.

Per-core decomposition:
  - x^T resident in SBUF; transposed projections with HOST-side weight column
    layouts:
      * Q pair lhsT (even h) = [Wq_h | Wq_h+1] -> PSUM [q_h; q_h+1]
      * K lhsT               = [Wk2_h | Wk1_h] -> PSUM [k2_h; k1_h]
    plus pe-table projections for the relative-position terms.
  - Speaker-select folded into an extended 256-dim score contraction:
      score^T[k,q] = [q*sq; q*(1-sq)] . [KA; KB] + [qp*sq; qp*(1-sq)] . [KPA; KPB]
    where KA = sk?k1:k2, KB = sk?k2:k1 (ACT copy + DVE copy_predicated from
    the interleaved PSUM into bf16 key tiles), q-side masks applied by DVE
    bf16 muls after an ACT psum->bf16 eviction.
  - Causal: score matmuls and exp sliced to [r:512] on diagonal tiles; one
    gpsimd affine_select (full width) zeroes k > q and the unwritten prefix.
  - Softmax denominators: ones-column (scaled by umask) appended to V in the
    PV lhsT -> row 64 of the PV PSUM holds the per-query sums. umask also
    scales V rows (exactly reproduces the reference key masking).
  - Normalize via reciprocal + PE outer-product broadcast, written shifted
    into the packed FC lhsT; final FC matmul + DMA out.
Host sums the two head-group partial outputs per batch.
"""
import sys

sys.path.insert(0, "/opt/trn_rl_repo")

import numpy as np

D = 1024
L = 1024
B = 4
DH = 64
NH = 8          # heads per core
NCORES = 8

_cached = {}


def _pe_table():
    num = 1201
    half = DH // 2
    freq = np.exp(np.arange(half, dtype=np.float32) * (-np.log(10000.0) / (half - 1)))
    pos_vals = np.arange(-num // 2, num // 2, dtype=np.float32)
    ang = pos_vals[:, None] * freq[None, :]
    table = np.concatenate([np.sin(ang), np.cos(ang)], axis=1).astype(np.float32)
    table[0] = 0.0
    idx = np.arange(-(L // 2), L // 2) + (num // 2 + 1)
    return table[idx]  # [L, DH] float32


def _build_program(nrep=1, loop=None):
    import concourse.bass as bass
    import concourse.mybir as mybir
    import concourse.tile as tile
    from concourse import bacc

    f32 = mybir.dt.float32
    f32r = mybir.dt.float32r
    bf16 = mybir.dt.bfloat16
    u8 = mybir.dt.uint8
    Exp = mybir.ActivationFunctionType.Exp
    Copy = mybir.ActivationFunctionType.Copy

    nc = bacc.Bacc(None, target_bir_lowering=False, debug=False)

    XT = nc.declare_dram_parameter("XT", [D, L], bf16, isOutput=False)
    WQK = nc.declare_dram_parameter("WQK", [NH, 2, D, 128], bf16, isOutput=False)
    WPOS = nc.declare_dram_parameter("WPOS", [NH, 2, DH, 128], bf16, isOutput=False)
    WV = nc.declare_dram_parameter("WV", [D, 512], bf16, isOutput=False)
    WFC = nc.declare_dram_parameter("WFC", [512, D], bf16, isOutput=False)
    PET = nc.declare_dram_parameter("PET", [DH, L], bf16, isOutput=False)
    MCM = nc.declare_dram_parameter("MCM", [128, L], bf16, isOutput=False)
    MCM2 = nc.declare_dram_parameter("MCM2", [128, L], bf16, isOutput=False)
    SKM = nc.declare_dram_parameter("SKM", [128, L], u8, isOutput=False)
    ONES1 = nc.declare_dram_parameter("ONES1", [1, 128], f32r, isOutput=False)
    UMASKT = nc.declare_dram_parameter("UMASKT", [128, 8], f32, isOutput=False)
    OCOLREP = nc.declare_dram_parameter("OCOLREP", [128, 64], bf16, isOutput=False)
    Y = nc.declare_dram_parameter("Y", [L, D], f32, isOutput=True)

    with tile.TileContext(nc) as tc:
        with tc.tile_pool(name="const", bufs=1) as const, \
             tc.tile_pool(name="wstream", bufs=3) as wstream, \
             tc.tile_pool(name="qk2", bufs=3) as qk2, \
             tc.tile_pool(name="qk3", bufs=4) as qk3, \
             tc.tile_pool(name="qev", bufs=4) as qev, \
             tc.tile_pool(name="exps", bufs=16) as exps, \
             tc.tile_pool(name="small", bufs=2) as small, \
             tc.tile_pool(name="yt", bufs=4) as ytp, \
             tc.tile_pool(name="proj_ps", bufs=3, space="PSUM") as proj_ps, \
             tc.tile_pool(name="score_ps", bufs=3, space="PSUM") as score_ps, \
             tc.tile_pool(name="pv_ps", bufs=2, space="PSUM") as pv_ps:

            # ---- resident constants; DMAs spread across engine queues and
            # ordered so head-0 can start ASAP ----
            xt = []
            for k in range(8):
                t = const.tile([128, L], bf16, tag=f"xt{k}")
                xt.append(t)
            dmaq = [nc.sync, nc.gpsimd, nc.sync, nc.sync]
            import contextlib
            loop_ctx = tc.For_i(0, loop, 1) if loop else contextlib.nullcontext()
            with loop_ctx:
              for _rep in range(nrep):
                  wpre = {}
                  wq0 = wstream.tile([128, D], bf16, tag="wq")
                  nc.sync.dma_start(wq0[:].rearrange("p (k c) -> p k c", c=128),
                                    WQK[0, 0].rearrange("(k p) c -> p k c", p=128))
                  wpre[("wq", 0)] = wq0
                  wk0 = wstream.tile([128, D], bf16, tag="wk")
                  nc.gpsimd.dma_start(wk0[:].rearrange("p (k c) -> p k c", c=128),
                                      WQK[0, 1].rearrange("(k p) c -> p k c", p=128))
                  wpre[("wk", 0)] = wk0
                  nc.sync.dma_start(xt[0][:], XT[0:128, :])
                  nc.sync.dma_start(xt[1][:], XT[128:256, :])
                  for k in range(2, 8):
                      dmaq[k % 4].dma_start(xt[k][:], XT[k * 128:(k + 1) * 128, :])
                  pet = const.tile([DH, L], bf16, tag="pet")
                  nc.gpsimd.dma_start(pet[:], PET[:])
                  mcm = const.tile([128, L], bf16, tag="mcm")
                  nc.sync.dma_start(mcm[:], MCM[:])
                  mcm2 = const.tile([128, L], bf16, tag="mcm2")
                  nc.sync.dma_start(mcm2[:], MCM2[:])
                  skm = const.tile([128, L], u8, tag="skm")
                  nc.gpsimd.dma_start(skm[:], SKM[:])
                  wqp, wkp = [], []
                  for h in range(NH):
                      if h % 2 == 0:
                          t0 = const.tile([DH, 128], bf16, tag=f"wqp{h}")
                          dmaq[h % 4].dma_start(t0[:], WPOS[h, 0])
                          wqp.append(t0)
                      else:
                          wqp.append(None)
                      t1 = const.tile([DH, 128], bf16, tag=f"wkp{h}")
                      dmaq[(h + 2) % 4].dma_start(t1[:], WPOS[h, 1])
                      wkp.append(t1)
                  ones1 = const.tile([1, 128], f32r, tag="ones1")
                  nc.sync.dma_start(ones1[:], ONES1[:])
                  umaskt = const.tile([128, 8], f32, tag="umaskt")
                  nc.sync.dma_start(umaskt[:], UMASKT[:])
                  vext = []
                  for tcn in range(8):
                      t = const.tile([128, NH * 65], bf16, tag=f"vext{tcn}")
                      vext.append(t)
                  outn = []
                  for g in range(4):
                      t = const.tile([128, L], bf16, tag=f"outn{g}")
                      outn.append(t)

                  hstate = {}
                  wcur = {}

                  def emit_proj_nt(h, nt):
                      ntsl = bass.ts(nt, 512)
                      if h % 2 == 0:
                          if nt == 0:
                              if ("wq", h) in wpre:
                                  wcur[("wq", h)] = wpre.pop(("wq", h))
                              else:
                                  wq_t = wstream.tile([128, D], bf16, tag="wq")
                                  nc.sync.dma_start(
                                      wq_t[:].rearrange("p (k c) -> p k c", c=128),
                                      WQK[h, 0].rearrange("(k p) c -> p k c", p=128))
                                  wcur[("wq", h)] = wq_t
                              qsd0 = qk3.tile([128, L], bf16, tag="qsd")
                              qsd1 = qk3.tile([128, L], bf16, tag="qsd")
                              qpsd0 = qk3.tile([128, L], bf16, tag="qpsd")
                              qpsd1 = qk3.tile([128, L], bf16, tag="qpsd")
                              hstate[("q", h)] = (qsd0, qpsd0)
                              hstate[("q", h + 1)] = (qsd1, qpsd1)
                          wq_t = wcur[("wq", h)]
                          qsd0, qpsd0 = hstate[("q", h)]
                          qsd1, qpsd1 = hstate[("q", h + 1)]
                          psq = proj_ps.tile([128, 512], f32, tag="proj")
                          for k in range(8):
                              nc.tensor.matmul(psq[:], wq_t[:, k * 128:(k + 1) * 128],
                                               xt[k][:, ntsl],
                                               start=(k == 0), stop=(k == 7))
                          qh = qev.tile([128, 512], bf16, tag="qh")
                          nc.scalar.copy(qh[:], psq[:])
                          nc.vector.tensor_mul(qsd0[0:64, ntsl], qh[0:64, :],
                                               mcm[0:64, ntsl])
                          nc.vector.tensor_mul(qsd0[64:128, ntsl], qh[0:64, :],
                                               mcm2[0:64, ntsl])
                          nc.vector.tensor_mul(qsd1[0:64, ntsl], qh[64:128, :],
                                               mcm2[64:128, ntsl])
                          nc.vector.tensor_mul(qsd1[64:128, ntsl], qh[64:128, :],
                                               mcm[64:128, ntsl])
                          psqp = proj_ps.tile([128, 512], f32, tag="proj")
                          nc.tensor.matmul(psqp[:], wqp[h][:], pet[:, ntsl],
                                           start=True, stop=True)
                          qph = qev.tile([128, 512], bf16, tag="qph")
                          nc.scalar.copy(qph[:], psqp[:])
                          nc.vector.tensor_mul(qpsd0[0:64, ntsl], qph[0:64, :],
                                               mcm[0:64, ntsl])
                          nc.vector.tensor_mul(qpsd0[64:128, ntsl], qph[0:64, :],
                                               mcm2[0:64, ntsl])
                          nc.vector.tensor_mul(qpsd1[0:64, ntsl], qph[64:128, :],
                                               mcm2[64:128, ntsl])
                          nc.vector.tensor_mul(qpsd1[64:128, ntsl], qph[64:128, :],
                                               mcm[64:128, ntsl])

                      # K / KP interleaved projections + blends (per head)
                      if nt == 0:
                          if ("wk", h) in wpre:
                              wcur[("wk", h)] = wpre.pop(("wk", h))
                          else:
                              wk_t = wstream.tile([128, D], bf16, tag="wk")
                              nc.sync.dma_start(
                                  wk_t[:].rearrange("p (k c) -> p k c", c=128),
                                  WQK[h, 1].rearrange("(k p) c -> p k c", p=128))
                              wcur[("wk", h)] = wk_t
                          k1t = qk2.tile([128, L], bf16, tag="k1t")
                          k2t = qk2.tile([128, L], bf16, tag="k2t")
                          hstate[("k", h)] = (k1t, k2t)
                      wk_t = wcur[("wk", h)]
                      k1t, k2t = hstate[("k", h)]
                      psk = proj_ps.tile([128, 512], f32, tag="proj")
                      for k in range(8):
                          nc.tensor.matmul(psk[:], wk_t[:, k * 128:(k + 1) * 128],
                                           xt[k][:, ntsl], start=(k == 0), stop=(k == 7))
                      nc.scalar.copy(k1t[:, ntsl], psk[:])
                      nc.vector.copy_predicated(k1t[0:64, ntsl], skm[64:128, ntsl],
                                                k1t[64:128, ntsl])
                      nc.vector.copy_predicated(k1t[64:128, ntsl], skm[64:128, ntsl],
                                                psk[0:64, :])

                      pskp = proj_ps.tile([128, 512], f32, tag="proj")
                      nc.tensor.matmul(pskp[:], wkp[h][:], pet[:, ntsl],
                                       start=True, stop=True)
                      nc.scalar.copy(k2t[:, ntsl], pskp[:])
                      nc.vector.copy_predicated(k2t[0:64, ntsl], skm[64:128, ntsl],
                                                k2t[64:128, ntsl])
                      nc.vector.copy_predicated(k2t[64:128, ntsl], skm[64:128, ntsl],
                                                pskp[0:64, :])

                  def emit_attn_qt(h, qt):
                      qsd, qpsd = hstate[("q", h)]
                      k1t, k2t = hstate[("k", h)]
                      qtsl = bass.ts(qt, 512)
                      jmax = 4 * (qt + 1)
                      ets = []
                      for j in range(jmax):
                          sps = score_ps.tile([128, 512], f32, tag="s")
                          et = exps.tile([128, 512], bf16, tag="e")
                          r = j * 128 - qt * 512
                          if r < 0:
                              nc.tensor.matmul(sps[:], k1t[:, j * 128:(j + 1) * 128],
                                               qsd[:, qtsl], start=True, stop=False)
                              nc.tensor.matmul(sps[:], k2t[:, j * 128:(j + 1) * 128],
                                               qpsd[:, qtsl], start=False, stop=True)
                              nc.scalar.activation(et[:], sps[:], Exp)
                          else:
                              qs = qt * 512
                              nc.tensor.matmul(sps[:, r:512],
                                               k1t[:, j * 128:(j + 1) * 128],
                                               qsd[:, qs + r:qs + 512],
                                               start=True, stop=False)
                              nc.tensor.matmul(sps[:, r:512],
                                               k2t[:, j * 128:(j + 1) * 128],
                                               qpsd[:, qs + r:qs + 512],
                                               start=False, stop=True)
                              nc.scalar.activation(et[:, r:512], sps[:, r:512], Exp)
                              # keep where y - x - r >= 0 (q >= k), else 0
                              nc.gpsimd.affine_select(
                                  out=et[:], in_=et[:],
                                  compare_op=mybir.AluOpType.is_ge,
                                  fill=0.0, base=-r,
                                  pattern=[[1, 512]], channel_multiplier=-1)
                          ets.append(et)
                      pvps = pv_ps.tile([65, 512], f32, tag="pv")
                      for j in range(jmax):
                          r = max(j * 128 - qt * 512, 0)
                          nc.tensor.matmul(pvps[:, r:512],
                                           vext[j][:, h * 65:(h + 1) * 65],
                                           ets[j][:, r:512],
                                           start=(j == 0), stop=(j == jmax - 1))
                      rc = small.tile([1, 512], f32, tag="rc")
                      nc.vector.reciprocal(rc[:], pvps[64:65, :])
                      rcr = small.tile([1, 512], f32r, tag="rcr")
                      nc.vector.tensor_copy(rcr[:], rc[:])
                      hstate[("n", h, qt)] = (pvps, rcr)

                  def emit_norm(h):
                      hstate.pop(("q", h))
                      hstate.pop(("k", h))
                      for qt in range(2):
                          pvps, rcr = hstate.pop(("n", h, qt))
                          qtsl = bass.ts(qt, 512)
                          bps = score_ps.tile([64, 512], f32, tag="s")
                          nc.tensor.matmul(bps[:], ones1[:, 0:64], rcr[:],
                                           start=True, stop=True)
                          bsb = small.tile([64, 512], f32, tag="bsb")
                          nc.scalar.copy(bsb[:], bps[:])
                          g, row0 = h // 2, (h % 2) * 64
                          nc.vector.tensor_mul(outn[g][row0:row0 + 64, qtsl],
                                               pvps[0:64, :], bsb[:])

                  emit_proj_nt(0, 0)
                  emit_proj_nt(0, 1)

                  # ---- V phase (needs xt + wv; emitted after proj(0) so head-0
                  # scores are not delayed behind the wv DMA) ----
                  wv = []
                  for k in range(8):
                      t = const.tile([128, 512], bf16, tag=f"wv{k}")
                      dmaq[k % 4].dma_start(t[:], WV[k * 128:(k + 1) * 128, :])
                      wv.append(t)
                  ocolrep = const.tile([128, 64], bf16, tag="ocolrep")
                  nc.sync.dma_start(ocolrep[:], OCOLREP[:])
                  for tcn in range(8):
                      ocols = vext[tcn][:].rearrange("p (h c) -> p h c", c=65)[:, :, 64]
                      nc.sync.dma_start(ocols, OCOLREP[:, tcn * 8:(tcn + 1) * 8])
                      pool = proj_ps if tcn % 2 == 0 else score_ps
                      psv = pool.tile([128, 512], f32, tag="proj" if tcn % 2 == 0 else "s")
                      for k in range(8):
                          nc.tensor.matmul(psv[:], xt[k][:, tcn * 128:(tcn + 1) * 128],
                                           wv[k][:], start=(k == 0), stop=(k == 7))
                      vslots = vext[tcn][:].rearrange(
                          "p (h c) -> p h c", c=65)[:, :, 0:64]
                      nc.scalar.activation(
                          vslots, psv[:].rearrange("p (h c) -> p h c", c=64),
                          Copy, scale=umaskt[:, tcn:tcn + 1])

                  for h in range(1, NH):
                      emit_proj_nt(h, 0)
                      emit_attn_qt(h - 1, 0)
                      emit_proj_nt(h, 1)
                      emit_attn_qt(h - 1, 1)
                      emit_norm(h - 1)
                  # wfc loads start as soon as the last head's W slots free up
                  wfc = []
                  for kc in range(4):
                      t = wstream.tile([128, D], bf16, tag=("wq" if kc % 2 == 0 else "wk"))
                      nc.sync.dma_start(t[:], WFC[kc * 128:(kc + 1) * 128, :])
                      wfc.append(t)
                  emit_attn_qt(NH - 1, 0)
                  emit_attn_qt(NH - 1, 1)
                  emit_norm(NH - 1)

                  # ---- FC (alternate PSUM pools to avoid eviction stalls) ----
                  for tcn in range(8):
                      tsl = bass.ts(tcn, 128)
                      for ct in range(2):
                          ctsl = bass.ts(ct, 512)
                          i3 = (tcn * 2 + ct) % 3
                          pool = (score_ps, proj_ps, pv_ps)[i3]
                          yps = pool.tile([128, 512], f32, tag=("s", "proj", "pv")[i3])
                          for kc in range(4):
                              nc.tensor.matmul(yps[:], outn[kc][:, tsl],
                                               wfc[kc][:, ctsl],
                                               start=(kc == 0), stop=(kc == 3))
                          yt = ytp.tile([128, 512], f32, tag="y")
                          if (tcn * 2 + ct) % 2 == 0:
                              nc.vector.tensor_copy(yt[:], yps[:])
                          else:
                              nc.scalar.copy(yt[:], yps[:])
                          dmaq[(tcn * 2 + ct) % 4].dma_start(
                              Y[tcn * 128:(tcn + 1) * 128, ct * 512:(ct + 1) * 512], yt[:])

    nc.compile()
    return nc


def _host_inputs(embed, umask, qmask, W_qkv, W_pos, W_fc):
    import ml_dtypes
    bf16 = ml_dtypes.bfloat16
    pe = _pe_table()
    pet = np.ascontiguousarray(pe.T).astype(bf16)  # [DH, L]
    ones1 = np.ones((1, 128), np.float32)
    in_maps = []
    for core in range(NCORES):
        b, hg = core // 2, core % 2
        sq = qmask[b].astype(np.float32)          # [L] in {0,1}
        um = umask[b].astype(np.float32)          # [L]
        mcm = np.empty((128, L), np.float32)
        mcm[0:64] = sq[None, :]
        mcm[64:128] = (1.0 - sq)[None, :]
        mcm2 = np.empty((128, L), np.float32)
        mcm2[0:64] = (1.0 - sq)[None, :]
        mcm2[64:128] = sq[None, :]
        skm = np.broadcast_to(qmask[b].astype(np.uint8)[None, :], (128, L)).copy()
        wqk = np.zeros((NH, 2, D, 128), np.float32)
        wpos = np.zeros((NH, 2, DH, 128), np.float32)
        for h in range(NH):
            gh = hg * NH + h
            k1c = W_qkv[:, 1 * D + gh * DH: 1 * D + (gh + 1) * DH]
            k2c = W_qkv[:, 2 * D + gh * DH: 2 * D + (gh + 1) * DH]
            wqk[h, 1] = np.concatenate([k2c, k1c], axis=1)
            kp1c = W_pos[:, 1 * D + gh * DH: 1 * D + (gh + 1) * DH]
            kp2c = W_pos[:, 2 * D + gh * DH: 2 * D + (gh + 1) * DH]
            wpos[h, 1] = np.concatenate([kp2c, kp1c], axis=1)
            if h % 2 == 0:
                q0 = W_qkv[:, 0 * D + gh * DH: 0 * D + (gh + 1) * DH]
                q1 = W_qkv[:, 0 * D + (gh + 1) * DH: 0 * D + (gh + 2) * DH]
                wqk[h, 0] = np.concatenate([q0, q1], axis=1)
                qp0 = W_pos[:, 0 * D + gh * DH: 0 * D + (gh + 1) * DH]
                qp1 = W_pos[:, 0 * D + (gh + 1) * DH: 0 * D + (gh + 2) * DH]
                wpos[h, 0] = np.concatenate([qp0, qp1], axis=1)
        umaskt = um.reshape(8, 128).T.copy()                     # [128, 8]
        ocolrep = np.repeat(umaskt[:, :, None], 8, axis=2).reshape(128, 64)
        in_maps.append({
            "XT": np.ascontiguousarray(embed[b].T).astype(bf16),
            "WQK": wqk.astype(bf16),
            "WPOS": wpos.astype(bf16),
            "WV": np.ascontiguousarray(
                W_qkv[:, 3 * D + hg * 512: 3 * D + (hg + 1) * 512]).astype(bf16),
            "WFC": np.ascontiguousarray(W_fc[hg * 512:(hg + 1) * 512, :]).astype(bf16),
            "PET": pet,
            "MCM": mcm.astype(bf16),
            "MCM2": mcm2.astype(bf16),
            "SKM": skm,
            "ONES1": ones1,
            "UMASKT": umaskt,
            "OCOLREP": np.ascontiguousarray(ocolrep).astype(bf16),
        })
    return in_maps


def kernel(embed, umask, qmask, W_qkv, W_pos, W_fc):
    from concourse.bass_utils import run_bass_kernel_spmd

    embed = np.asarray(embed, dtype=np.float32)
    umask = np.asarray(umask)
    qmask = np.asarray(qmask)
    W_qkv = np.asarray(W_qkv, dtype=np.float32)
    W_pos = np.asarray(W_pos, dtype=np.float32)
    W_fc = np.asarray(W_fc, dtype=np.float32)

    if "nc" not in _cached:
        _cached["nc"] = _build_program()
    nc = _cached["nc"]

    in_maps = _host_inputs(embed, umask, qmask, W_qkv, W_pos, W_fc)
    res = run_bass_kernel_spmd(nc, in_maps, list(range(NCORES))).results

    y = np.empty((B, L, D), np.float32)
    for b in range(B):
        y[b] = res[2 * b]["Y"] + res[2 * b + 1]["Y"]
    return y
